# revision 57
# baseline (speedup 1.0000x reference)
"""Multi-head attention (B=2, N=2048, d_model=1024, 16 heads x 64) on 8
Trainium2 NeuronCores.

Sharding: batch x head-group. Core c handles batch b = c//4 and heads
4*(c%4) .. 4*(c%4)+3. Projection weights are column-sliced (rows for Wo) so
each core computes q/k/v projections only for its 4 heads, full attention
for those heads, and a partial output projection. The host sums the four
partial outputs per batch (tensor-parallel reduce on to_out) and adds bo.

All matmul operands are fp16 (PSUM accumulate stays fp32): same PE
row-stream rate as fp32r but half the LDWEIGHTS install time, half the
DMA bytes, and 2x DVE rates on 16-bit evictions.

Device kernel (per core):
  warm  : dummy matmuls on a zero tile raise the PE out of its low
          p-state while the first input DMAs land (~12us first-DMA
          ring latency); all x-block DMAs are pre-issued on the gpsimd
          queue through a 3-deep ring so transfers prefetch in
          consumption order
  qT/kT : projections producing [head-dim, seq] (lhsT = W chunk)
  v     : natural [seq, head-dim] with a ones column folded in (M=65),
          K=128 contraction chunks; storage padded to 66 cols so every
          head slice starts 4-byte aligned (2-byte-aligned fp16
          LDWEIGHTS corrupts nondeterministically); ones written by
          memset (the strided DMA degenerates to 8K 2-byte descriptors)
  ST    : k^T q per head -> scores^T [keys, queries]; K=64 row-tile PAIRS
          (two heads on PE tiles T0/T8)
  E     : exp(ST * scale) via ScalarE eviction PSUM->SBUF -> fp16 (the
          wall: ~1 elem/lane/cycle regardless of dtype)
  AV    : [v|ones]^T @ E -> [65, 512]: rows 0-63 = O^T, row 64 = denom
  norm  : reciprocal_approx_fast (single custom-DVE op, ~5x faster than
          reciprocal) on den4, fp16 cast, K=4 pattern matmul broadcasts
          the per-query recip across the 64 output lanes, DVE multiply
          into O^T  (gpsimd partition_broadcast would be cheaper but
          silently produces garbage on HW in this compile path)
  out   : O^T-as-lhsT @ Wo slice -> partial [2048, 1024] written fp16;
          the last query block's finalize is split into query halves
          interleaved with its out-proj chunks, which run on the freed
          AV PSUM banks (4-slot ring) with ScalarE evictions and the
          final writes spread over all three DMA queues
"""

import numpy as np

import concourse.mybir as mybir
import concourse.tile as tile
from concourse import bacc
from concourse import bass_utils
from concourse.tile_rust import add_dep_helper

F32 = mybir.dt.float32
F16 = mybir.dt.float16
EXP = mybir.ActivationFunctionType.Exp

B = 2
N = 2048
D_MODEL = 1024
NHEAD = 16
DIM_HEAD = 64
SCALE = DIM_HEAD ** (-0.5)
N_CORES = 8
HEADS_PER_CORE = 4          # 2 pairs
INNER = HEADS_PER_CORE * DIM_HEAD  # 256

QB = 512                    # query block
N_QB = N // QB              # 4
N_KC = N // 128             # 16 key chunks


def build_nc():
    nc = bacc.Bacc("TRN2", target_bir_lowering=False, debug=False,
                   num_devices=N_CORES)
    xqt = nc.dram_tensor("xqt", [D_MODEL, N], F16, kind="ExternalInput").ap()
    xkt = nc.dram_tensor("xkt", [D_MODEL, N], F16, kind="ExternalInput").ap()
    xvt = nc.dram_tensor("xvt", [D_MODEL, N], F16, kind="ExternalInput").ap()
    wq = nc.dram_tensor("wq", [D_MODEL, INNER], F16, kind="ExternalInput").ap()
    wk = nc.dram_tensor("wk", [D_MODEL, INNER], F16, kind="ExternalInput").ap()
    wv = nc.dram_tensor("wv", [D_MODEL, INNER], F16, kind="ExternalInput").ap()
    wo = nc.dram_tensor("wo", [INNER, D_MODEL], F16, kind="ExternalInput").ap()
    # bc pattern: pat2[k, m] = 1 at (0, m<64) and (32, m>=64) — broadcasts the
    # two denominator-reciprocal rows across the pair's 128 output lanes
    pat2 = nc.dram_tensor("pat2", [128, 128], F16, kind="ExternalInput").ap()
    out = nc.dram_tensor("out", [N, D_MODEL], F16, kind="ExternalOutput").ap()

    with tile.TileContext(nc) as tc:
        with (
            tc.tile_pool(name="wpool", bufs=1) as wpool,
            tc.tile_pool(name="persist", bufs=1) as persist,
            tc.tile_pool(name="xin", bufs=3) as xin,
            tc.tile_pool(name="ering", bufs=9) as ering,
            tc.tile_pool(name="stage", bufs=3) as stage,
            tc.tile_pool(name="ps_st", bufs=2, space="PSUM") as ps_st,
            tc.tile_pool(name="ps_av", bufs=1, space="PSUM") as ps_av,
            tc.tile_pool(name="ps_misc", bufs=2, space="PSUM") as ps_misc,
        ):
            # ---- weights on sync queue, ordered by first use ----
            wk_sb = wpool.tile([128, 8, INNER], F16)
            nc.sync.dma_start(wk_sb[:], wk.rearrange("(c p) m -> p c m", p=128))
            wq_sb = wpool.tile([128, 8, INNER], F16)
            nc.sync.dma_start(wq_sb[:], wq.rearrange("(c p) m -> p c m", p=128))

            # ---- PE p-state warmup on a zeroed tile while DMAs land ----
            warm = wpool.tile([128, 512], F16)
            nc.vector.memset(warm[:], 0.0)
            for i in range(12):
                wp = ps_misc.tile([128, 512], F32, tag="mp", name=f"warm{i}")
                nc.tensor.matmul(wp[:], warm[:, 0:128], warm[:],
                                 start=True, stop=True)
            for i in range(20):
                wp = ps_misc.tile([128, 256], F32, tag="mp",
                                  padded_shape=[128, 512], name=f"warms{i}")
                nc.tensor.matmul(wp[:], warm[:, 0:128], warm[:, 0:256],
                                 start=True, stop=True)

            qt_sb = persist.tile([128, 2, N], F16)
            kt_sb = persist.tile([128, 2, N], F16)
            v_sb = persist.tile([128, N_KC, HEADS_PER_CORE, DIM_HEAD], F16)
            ot_sb = persist.tile([128, 2, N], F16)
            # ones lhsT for the denominator matmuls; den4 rows 1-31 stay at
            # the memset 1.0 so the full-tile reciprocal/cast never see
            # garbage (0 x inf = NaN would poison the bc accumulation)
            ones_sb = wpool.tile([128, 1], F16)
            nc.vector.memset(ones_sb[:], 1.0)
            den4 = wpool.tile([128, 512], F32)
            nc.vector.memset(den4[:], 1.0)

            xqt_r = xqt.rearrange("(c p) n -> p c n", p=128)
            xkt_r = xkt.rearrange("(c p) n -> p c n", p=128)
            xvt_r = xvt.rearrange("(c p) n -> p c n", p=128)

            def emit_kt_m(xk_t, n, m):
                ns = slice(n * QB, (n + 1) * QB)
                pk = ps_misc.tile([128, QB], F32, tag="mp", name=f"pk{n}{m}")
                for c in range(8):
                    nc.tensor.matmul(
                        pk[:], wk_sb[:, c, m * 128:(m + 1) * 128],
                        xk_t[:, c, :], start=(c == 0), stop=(c == 7))
                nc.vector.tensor_copy(kt_sb[:, m, ns], pk[:])

            def emit_kt(n, ms=(0, 1)):
                xk_t = xin_tiles[('k', n)]
                for m in ms:
                    emit_kt_m(xk_t, n, m)
                return xk_t

            def emit_qt_m(xq_t, n, m):
                ns = slice(n * QB, (n + 1) * QB)
                pq = ps_misc.tile([128, QB], F32, tag="mp", name=f"pq{n}{m}")
                for c in range(8):
                    nc.tensor.matmul(
                        pq[:], wq_sb[:, c, m * 128:(m + 1) * 128],
                        xq_t[:, c, :], start=(c == 0), stop=(c == 7))
                nc.vector.tensor_copy(qt_sb[:, m, ns], pq[:])

            def emit_qt(n, ms=(0, 1)):
                xq_t = xin_tiles[('q', n)]
                for m in ms:
                    emit_qt_m(xq_t, n, m)
                return xq_t

            def emit_vblock(n):
                xv_t = xin_tiles[('v', n)]
                for kci in range(4):
                    kc = n * 4 + kci
                    kcs = slice(kci * 128, (kci + 1) * 128)
                    pv = ps_misc.tile([128, INNER], F32, tag="mp",
                                      padded_shape=[128, 512], name=f"pv{kc}")
                    for c in range(8):
                        nc.tensor.matmul(
                            pv[:], xv_t[:, c, kcs],
                            wv_sb[:, c, :], start=(c == 0), stop=(c == 7))
                    nc.vector.tensor_copy(
                        v_sb[:, kc, :, 0:DIM_HEAD],
                        pv[:].rearrange("p (h d) -> p h d", h=HEADS_PER_CORE))

            def emit_outproj_chunk(qb, idx, tail=False):
                qc = qb * 4 + idx // 2
                dc = idx % 2
                cs = slice(qc * 128, (qc + 1) * 128)
                if tail and idx % 3 == 2:
                    # after the last evict_pair the AV PSUM bank is free:
                    # widen the tail ring to 3 slots so the out-projection
                    # matmuls don't serialize behind evictions
                    op = ps_av.tile([128, 512], F32, tag="av0",
                                    name=f"op{qc}{dc}")
                else:
                    op = ps_misc.tile([128, 512], F32, tag="mp",
                                      name=f"op{qc}{dc}")
                for ic in range(2):
                    nc.tensor.matmul(
                        op[:], ot_sb[:, ic, cs],
                        wo_sb[:, ic, dc * 512:(dc + 1) * 512],
                        start=(ic == 0), stop=(ic == 1))
                o_stage = stage.tile([128, 512], F16,
                                     tag="ostaget" if tail else "ostage",
                                     name=f"ost{qc}{dc}", bufs=4 if tail else 2)
                if tail:
                    nc.scalar.activation(o_stage[:], op[:],
                                         mybir.ActivationFunctionType.Copy)
                    # spread the final writes over all three DMA queues so
                    # the drain isn't serialized on one ring
                    q = (nc.sync, nc.scalar, nc.gpsimd)[idx % 3]
                    q.dma_start(out[cs, dc * 512:(dc + 1) * 512], o_stage[:])
                else:
                    nc.vector.tensor_copy(o_stage[:], op[:])
                    nc.sync.dma_start(out[cs, dc * 512:(dc + 1) * 512],
                                      o_stage[:])

            def emit_st(qb, p, kc):
                qs = slice(qb * QB, (qb + 1) * QB)
                ks = slice(kc * 128, (kc + 1) * 128)
                st = ps_st.tile([128, 1024], F32, tag="st", name=f"st{qb}{p}{kc}")
                mm0 = nc.tensor.matmul(st[:, 0:512], kt_sb[0:64, p, ks],
                                       qt_sb[0:64, p, qs], start=True, stop=True)
                nc.tensor.matmul(st[:, 512:1024], kt_sb[64:128, p, ks],
                                 qt_sb[64:128, p, qs], start=True, stop=True)
                e_t = ering.tile([128, 2, 512], F16, tag="e",
                                 name=f"e{qb}{p}{kc}")
                nc.scalar.activation(
                    e_t[:], st[:].rearrange("p (h n) -> p h n", h=2),
                    EXP, scale=float(SCALE))
                return e_t, mm0

            def emit_av(qb, p, kc, st_, e_t):
                av_ps = st_["av"]
                # the two heads write disjoint 64-col PE tiles (positions
                # (0,0) and (0,64)) and stream CONCURRENTLY — M=65 with the
                # ones column would round the col tile to 128 and serialize
                for hh in range(2):
                    nc.tensor.matmul(
                        av_ps[hh * 64:(hh + 1) * 64, :],
                        v_sb[:, kc, 2 * p + hh, :], e_t[:, hh, :],
                        start=(kc == 0), stop=(kc == N_KC - 1),
                        skip_group_check=True)
                # denominator partials on the idle vector engines: two
                # independent serial chains (even kc on DVE, odd on GpSimd)
                eng = nc.vector if kc % 2 == 0 else nc.gpsimd
                esum = st_["esa"] if kc % 2 == 0 else st_["esb"]
                if kc < 2:
                    eng.tensor_copy(esum[:], e_t[:])
                else:
                    eng.tensor_add(esum[:], esum[:], e_t[:])

            def evict_pair(qb, p, st_, tail=False):
                av_ps = st_["av"]
                # denominators: contract the two esum partials per head into
                # a tiny PSUM tile (M=1 rows at col positions 0 / 32)
                den_ps = ps_av.tile([33, 512], F32, tag="den",
                                    name=f"dps{qb}_{p}")
                for hh in range(2):
                    ro = 32 * hh
                    nc.tensor.matmul(den_ps[ro:ro + 1, :], ones_sb[:, 0:1],
                                     st_["esa"][:, hh, :], start=True,
                                     stop=False, skip_group_check=True)
                    nc.tensor.matmul(den_ps[ro:ro + 1, :], ones_sb[:, 0:1],
                                     st_["esb"][:, hh, :], start=False,
                                     stop=True, skip_group_check=True)
                a_sb = stage.tile([128, 512], F32, tag="avsb",
                                  name=f"avsb{qb}_{p}", bufs=3)
                if tail:
                    # ScalarE is exp-idle at the tail: evict O^T there
                    # while DVE pulls the denom rows
                    nc.scalar.activation(a_sb[:], av_ps[:],
                                         mybir.ActivationFunctionType.Copy)
                else:
                    nc.vector.tensor_copy(a_sb[:], av_ps[:])
                nc.vector.tensor_copy(den4[0:1, :], den_ps[0:1, :])
                nc.vector.tensor_copy(den4[32:33, :], den_ps[32:33, :])
                return a_sb

            def finalize_pair(qb, p, avsb, order_after=None, halves=1,
                              after_half=None):
                qb0 = qb * QB
                rec = stage.tile([128, 512], F32, tag="rec",
                                 name=f"rec{qb}{p}", bufs=2)
                with nc.allow_low_precision(reason="softmax denom recip"):
                    nc.vector.reciprocal_approx_fast(rec[:], den4[:])
                recr = stage.tile([128, 512], F16, tag="recr",
                                  name=f"recr{qb}{p}", bufs=2)
                nc.vector.tensor_copy(recr[:], rec[:])
                bc = ps_misc.tile([128, 512], F32, tag="mp", name=f"bc{qb}{p}")
                hw = QB // halves
                for half in range(halves):
                    hs = slice(half * hw, (half + 1) * hw)
                    bcmm = nc.tensor.matmul(bc[:, hs], pat_sb[:, :],
                                            recr[:, hs], start=True, stop=True)
                    if order_after is not None:
                        add_dep_helper(order_after.ins, bcmm.ins, sync=False,
                                       reason="hold bc behind ST stream")
                for half in range(halves):
                    hs = slice(half * hw, (half + 1) * hw)
                    nc.vector.tensor_mul(
                        ot_sb[:, p, qb0 + half * hw:qb0 + (half + 1) * hw],
                        avsb[:, hs],
                        bc[:, hs])
                    if after_half is not None:
                        after_half(half)

            def new_phase_state(qb, p):
                return dict(
                    av=ps_av.tile([128, 512], F32, tag="av0",
                                  name=f"av_{qb}_{p}"),
                    esa=stage.tile([128, 2, 512], F16, tag="esa",
                                   name=f"esa{qb}{p}", bufs=2),
                    esb=stage.tile([128, 2, 512], F16, tag="esb",
                                   name=f"esb{qb}{p}", bufs=2),
                )

            def emit_late_weights():
                nc.sync.dma_start(wo_sb[:],
                                  wo.rearrange("(c p) d -> p c d", p=128))
                nc.sync.dma_start(pat_sb[:], pat2[:])

            def phase_fillers(qb, p):
                f = []
                if qb == 0 and p == 0:
                    f.append((5, emit_late_weights))
                    for n in range(1, N_QB):
                        f.append((4 * n - 1, lambda n=n: (emit_kt(n),
                                                          emit_vblock(n))))
                elif qb == 0 and p == 1:
                    f.append((7, lambda: emit_qt(1)))
                else:
                    prev = qb - 1
                    if p == 0:
                        for g in range(4):
                            f.append(((9, 11, 13, 15)[g],
                                      lambda g=g: emit_outproj_chunk(prev, g)))
                    else:
                        if qb < N_QB - 1:
                            f.append((5, lambda: emit_qt(qb + 1)))
                        for g in range(4):
                            f.append(((3, 7, 10, 13)[g],
                                      lambda g=g: emit_outproj_chunk(prev, 4 + g)))
                return dict(f)

            # pre-issue every input-block DMA in consumption order through
            # a deep xin ring so transfers prefetch ahead of the chains
            # that consume them
            wv_sb = wpool.tile([128, 8, INNER], F16)
            xin_tiles = {}
            xin_order = [('k', 0), ('q', 0), ('v', 0), ('k', 1), ('v', 1),
                         ('k', 2), ('v', 2), ('k', 3), ('v', 3),
                         ('q', 1), ('q', 2), ('q', 3)]
            xin_src = {'k': xkt_r, 'q': xqt_r, 'v': xvt_r}
            for kind, n in xin_order:
                t = xin.tile([128, 8, QB], F16, tag="xin",
                             name=f"x{kind}_{n}", bufs=4)
                ns = slice(n * QB, (n + 1) * QB)
                nc.gpsimd.dma_start(t[:], xin_src[kind][:, :, ns])
                xin_tiles[(kind, n)] = t
                if (kind, n) == ('k', 0):
                    nc.scalar.dma_start(
                        wv_sb[:], wv.rearrange("(c p) m -> p c m", p=128))

            emit_kt(0)
            emit_qt(0)

            wo_sb = wpool.tile([128, 2, D_MODEL], F16)
            pat_sb = wpool.tile([128, 128], F16)

            emit_vblock(0)

            AV_LAG = 4
            phases = [(qb, p) for qb in range(N_QB) for p in range(2)]
            pending = None      # (qb, p, avs, [(kc, e_t)...])
            pending_fin = None  # (qb, p, avsb)

            for qb, p in phases:
                avs = new_phase_state(qb, p)
                fillers = phase_fillers(qb, p)
                eq = []
                for kc in range(N_KC):
                    e_t, stmm = emit_st(qb, p, kc)
                    eq.append((kc, e_t))
                    if kc == AV_LAG - 1 and pending is not None:
                        pq, pp, pavs, peq = pending
                        for pkc, pe_t in peq:
                            emit_av(pq, pp, pkc, pavs, pe_t)
                        pending_fin = (pq, pp, evict_pair(pq, pp, pavs))
                        pending = None
                    if kc == 7 and pending_fin is not None:
                        fq, fp, favsb = pending_fin
                        finalize_pair(fq, fp, favsb, order_after=stmm)
                        pending_fin = None
                    if kc >= AV_LAG:
                        pkc, pe_t = eq[kc - AV_LAG]
                        emit_av(qb, p, pkc, avs, pe_t)
                    if kc in fillers:
                        fillers[kc]()
                pending = (qb, p, avs, eq[N_KC - AV_LAG:])

            pq, pp, pavs, peq = pending
            for pkc, pe_t in peq:
                emit_av(pq, pp, pkc, pavs, pe_t)
            # tail: normalize by query halves, interleaving the final
            # out-projection chunks so PE work overlaps the recip/mul chain
            finalize_pair(pq, pp, evict_pair(pq, pp, pavs, tail=True), halves=2,
                          after_half=lambda half: [
                              emit_outproj_chunk(N_QB - 1, 4 * half + g,
                                                 tail=True)
                              for g in range(4)])
    nc.compile()
    return nc


_NC_CACHE = None


def _get_nc():
    global _NC_CACHE
    if _NC_CACHE is None:
        _NC_CACHE = build_nc()
    return _NC_CACHE


def _make_pat2():
    pat = np.zeros((128, 128), np.float16)
    pat[0, 0:64] = 1.0
    pat[32, 64:128] = 1.0
    return pat


def make_in_maps(query, key, value, Wq, Wk, Wv, Wo):
    query = np.asarray(query, np.float32)
    key = np.asarray(key, np.float32)
    value = np.asarray(value, np.float32)
    pat2 = _make_pat2()
    in_maps = []
    for c in range(N_CORES):
        b = c // 4
        hg = c % 4
        cols = slice(hg * INNER, (hg + 1) * INNER)
        in_maps.append({
            "xqt": np.ascontiguousarray(query[b].T).astype(np.float16),
            "xkt": np.ascontiguousarray(key[b].T).astype(np.float16),
            "xvt": np.ascontiguousarray(value[b].T).astype(np.float16),
            "wq": np.asarray(Wq[:, cols]).astype(np.float16),
            "wk": np.asarray(Wk[:, cols]).astype(np.float16),
            "wv": np.asarray(Wv[:, cols]).astype(np.float16),
            "wo": np.asarray(Wo[cols, :]).astype(np.float16),
            "pat2": pat2,
        })
    return in_maps


def kernel(query, key, value, Wq, Wk, Wv, Wo, bo, _trace=False, _trace_cores=None):
    nc = _get_nc()
    in_maps = make_in_maps(query, key, value, Wq, Wk, Wv, Wo)
    res = bass_utils.run_bass_kernel_spmd(
        nc, in_maps, core_ids=list(range(N_CORES)), trace=_trace,
        trace_cores=_trace_cores)
    out = np.zeros((B, N, D_MODEL), np.float32)
    for c in range(N_CORES):
        out[c // 4] += res.results[c]["out"].astype(np.float32)
    out += np.asarray(bo, np.float32)[None, None, :]
    if _trace:
        return out, res
    return out


# revision 58
# speedup vs baseline: 1.2261x; 1.2261x over previous
"""Multi-head attention (B=2, N=2048, d_model=1024, 16 heads x 64) on 8
Trainium2 NeuronCores.

Sharding: batch x head-group. Core c handles batch b = c//4 and heads
4*(c%4) .. 4*(c%4)+3. Projection weights are column-sliced (rows for Wo) so
each core computes q/k/v projections only for its 4 heads, full attention
for those heads, and a partial output projection. The host sums the four
partial outputs per batch (tensor-parallel reduce on to_out) and adds bo.

All matmul operands are fp16 (PSUM accumulate stays fp32): same PE
row-stream rate as fp32r but half the LDWEIGHTS install time, half the
DMA bytes, and 2x DVE rates on 16-bit evictions.

Device kernel (per core):
  warm  : dummy matmuls on a zero tile raise the PE out of its low
          p-state while the first input DMAs land (~12us first-DMA
          ring latency); all x-block DMAs are pre-issued on the gpsimd
          queue through a 3-deep ring so transfers prefetch in
          consumption order
  qT/kT : projections producing [head-dim, seq] (lhsT = W chunk)
  v     : natural [seq, head-dim] with a ones column folded in (M=65),
          K=128 contraction chunks; storage padded to 66 cols so every
          head slice starts 4-byte aligned (2-byte-aligned fp16
          LDWEIGHTS corrupts nondeterministically); ones written by
          memset (the strided DMA degenerates to 8K 2-byte descriptors)
  ST    : k^T q per head -> scores^T [keys, queries]; K=64 row-tile PAIRS
          (two heads on PE tiles T0/T8)
  E     : exp(ST * scale) via ScalarE eviction PSUM->SBUF -> fp16 (the
          wall: ~1 elem/lane/cycle regardless of dtype)
  AV    : [v|ones]^T @ E -> [65, 512]: rows 0-63 = O^T, row 64 = denom
  norm  : reciprocal_approx_fast (single custom-DVE op, ~5x faster than
          reciprocal) on den4, fp16 cast, K=4 pattern matmul broadcasts
          the per-query recip across the 64 output lanes, DVE multiply
          into O^T  (gpsimd partition_broadcast would be cheaper but
          silently produces garbage on HW in this compile path)
  out   : O^T-as-lhsT @ Wo slice -> partial [2048, 1024] written fp16;
          the last query block's finalize is split into query halves
          interleaved with its out-proj chunks, which run on the freed
          AV PSUM banks (4-slot ring) with ScalarE evictions and the
          final writes spread over all three DMA queues
"""

import numpy as np

import concourse.mybir as mybir
import concourse.tile as tile
from concourse import bacc
from concourse import bass_utils
from concourse.tile_rust import add_dep_helper

F32 = mybir.dt.float32
F16 = mybir.dt.float16
EXP = mybir.ActivationFunctionType.Exp

B = 2
N = 2048
D_MODEL = 1024
NHEAD = 16
DIM_HEAD = 64
SCALE = DIM_HEAD ** (-0.5)
N_CORES = 8
HEADS_PER_CORE = 4          # 2 pairs
INNER = HEADS_PER_CORE * DIM_HEAD  # 256

QB = 512                    # query block
N_QB = N // QB              # 4
N_KC = N // 128             # 16 key chunks


def build_nc():
    nc = bacc.Bacc("TRN2", target_bir_lowering=False, debug=False,
                   num_devices=N_CORES)
    xqt = nc.dram_tensor("xqt", [D_MODEL, N], F16, kind="ExternalInput").ap()
    xkt = nc.dram_tensor("xkt", [D_MODEL, N], F16, kind="ExternalInput").ap()
    xvt = nc.dram_tensor("xvt", [D_MODEL, N], F16, kind="ExternalInput").ap()
    wq = nc.dram_tensor("wq", [D_MODEL, INNER], F16, kind="ExternalInput").ap()
    wk = nc.dram_tensor("wk", [D_MODEL, INNER], F16, kind="ExternalInput").ap()
    wv = nc.dram_tensor("wv", [D_MODEL, INNER], F16, kind="ExternalInput").ap()
    wo = nc.dram_tensor("wo", [INNER, D_MODEL], F16, kind="ExternalInput").ap()
    # bc pattern: pat4[k, p, m] = 1 where head k owns output rows m in pair p
    pat4 = nc.dram_tensor("pat4", [128, 2, 128], F16, kind="ExternalInput").ap()
    out = nc.dram_tensor("out", [N, D_MODEL], F16, kind="ExternalOutput").ap()

    with tile.TileContext(nc) as tc:
        with (
            tc.tile_pool(name="wpool", bufs=1) as wpool,
            tc.tile_pool(name="persist", bufs=1) as persist,
            tc.tile_pool(name="xin", bufs=3) as xin,
            tc.tile_pool(name="ering", bufs=9) as ering,
            tc.tile_pool(name="stage", bufs=3) as stage,
            tc.tile_pool(name="ps_st", bufs=2, space="PSUM") as ps_st,
            tc.tile_pool(name="ps_av", bufs=1, space="PSUM") as ps_av,
            tc.tile_pool(name="ps_misc", bufs=2, space="PSUM") as ps_misc,
        ):
            # ---- weights on sync queue, ordered by first use ----
            wk_sb = wpool.tile([128, 8, INNER], F16)
            nc.sync.dma_start(wk_sb[:], wk.rearrange("(c p) m -> p c m", p=128))
            wq_sb = wpool.tile([128, 8, INNER], F16)
            nc.sync.dma_start(wq_sb[:], wq.rearrange("(c p) m -> p c m", p=128))

            # ---- PE p-state warmup on a zeroed tile while DMAs land ----
            warm = wpool.tile([128, 512], F16)
            nc.vector.memset(warm[:], 0.0)
            for i in range(12):
                wp = ps_misc.tile([128, 512], F32, tag="mp", name=f"warm{i}")
                nc.tensor.matmul(wp[:], warm[:, 0:128], warm[:],
                                 start=True, stop=True)
            for i in range(20):
                wp = ps_misc.tile([128, 256], F32, tag="mp",
                                  padded_shape=[128, 512], name=f"warms{i}")
                nc.tensor.matmul(wp[:], warm[:, 0:128], warm[:, 0:256],
                                 start=True, stop=True)

            qt_sb = persist.tile([128, 2, N], F16)
            kt_sb = persist.tile([128, 2, N], F16)
            # last dim padded to 66 so each head's slice starts 4-byte
            # aligned (66*2 = 132 bytes); the matmul AP still reads 65 cols
            v_sb = persist.tile([128, N_KC, HEADS_PER_CORE, DIM_HEAD + 2], F16)
            ot_sb = persist.tile([128, 2, N], F16)

            xqt_r = xqt.rearrange("(c p) n -> p c n", p=128)
            xkt_r = xkt.rearrange("(c p) n -> p c n", p=128)
            xvt_r = xvt.rearrange("(c p) n -> p c n", p=128)

            def emit_kt_m(xk_t, n, m):
                ns = slice(n * QB, (n + 1) * QB)
                pk = ps_misc.tile([128, QB], F32, tag="mp", name=f"pk{n}{m}")
                for c in range(8):
                    nc.tensor.matmul(
                        pk[:], wk_sb[:, c, m * 128:(m + 1) * 128],
                        xk_t[:, c, :], start=(c == 0), stop=(c == 7))
                nc.vector.tensor_copy(kt_sb[:, m, ns], pk[:])

            def emit_kt(n, ms=(0, 1)):
                xk_t = xin_tiles[('k', n)]
                for m in ms:
                    emit_kt_m(xk_t, n, m)
                return xk_t

            def emit_qt_m(xq_t, n, m):
                ns = slice(n * QB, (n + 1) * QB)
                pq = ps_misc.tile([128, QB], F32, tag="mp", name=f"pq{n}{m}")
                for c in range(8):
                    nc.tensor.matmul(
                        pq[:], wq_sb[:, c, m * 128:(m + 1) * 128],
                        xq_t[:, c, :], start=(c == 0), stop=(c == 7))
                nc.vector.tensor_copy(qt_sb[:, m, ns], pq[:])

            def emit_qt(n, ms=(0, 1)):
                xq_t = xin_tiles[('q', n)]
                for m in ms:
                    emit_qt_m(xq_t, n, m)
                return xq_t

            def emit_vblock(n):
                xv_t = xin_tiles[('v', n)]
                for kci in range(4):
                    kc = n * 4 + kci
                    kcs = slice(kci * 128, (kci + 1) * 128)
                    pv = ps_misc.tile([128, INNER], F32, tag="mp",
                                      padded_shape=[128, 512], name=f"pv{kc}")
                    for c in range(8):
                        nc.tensor.matmul(
                            pv[:], xv_t[:, c, kcs],
                            wv_sb[:, c, :], start=(c == 0), stop=(c == 7))
                    nc.vector.tensor_copy(
                        v_sb[:, kc, :, 0:DIM_HEAD],
                        pv[:].rearrange("p (h d) -> p h d", h=HEADS_PER_CORE))

            def emit_outproj_chunk(qb, idx, tail=False):
                qc = qb * 4 + idx // 2
                dc = idx % 2
                cs = slice(qc * 128, (qc + 1) * 128)
                if tail and idx % 4 >= 2:
                    # after the last evict_pair the AV PSUM banks are free:
                    # widen the tail ring to 4 slots so the out-projection
                    # matmuls don't serialize behind evictions
                    op = ps_av.tile([128, 512], F32, tag=f"av{idx % 2}",
                                    name=f"op{qc}{dc}")
                else:
                    op = ps_misc.tile([128, 512], F32, tag="mp",
                                      name=f"op{qc}{dc}")
                for ic in range(2):
                    nc.tensor.matmul(
                        op[:], ot_sb[:, ic, cs],
                        wo_sb[:, ic, dc * 512:(dc + 1) * 512],
                        start=(ic == 0), stop=(ic == 1))
                o_stage = stage.tile([128, 512], F16,
                                     tag="ostaget" if tail else "ostage",
                                     name=f"ost{qc}{dc}", bufs=4 if tail else 2)
                if tail:
                    nc.scalar.activation(o_stage[:], op[:],
                                         mybir.ActivationFunctionType.Copy)
                    # spread the final writes over all three DMA queues so
                    # the drain isn't serialized on one ring
                    q = (nc.sync, nc.scalar, nc.gpsimd)[idx % 3]
                    q.dma_start(out[cs, dc * 512:(dc + 1) * 512], o_stage[:])
                else:
                    nc.vector.tensor_copy(o_stage[:], op[:])
                    nc.sync.dma_start(out[cs, dc * 512:(dc + 1) * 512],
                                      o_stage[:])

            qb_state = {}

            def emit_st(qb, p, kc):
                qs = slice(qb * QB, (qb + 1) * QB)
                ks = slice(kc * 128, (kc + 1) * 128)
                st = ps_st.tile([128, 1024], F32, tag="st", name=f"st{qb}{p}{kc}")
                mm0 = nc.tensor.matmul(st[:, 0:512], kt_sb[0:64, p, ks],
                                       qt_sb[0:64, p, qs], start=True, stop=True)
                nc.tensor.matmul(st[:, 512:1024], kt_sb[64:128, p, ks],
                                 qt_sb[64:128, p, qs], start=True, stop=True)
                e_t = ering.tile([128, 2, 512], F16, tag="e",
                                 name=f"e{qb}{p}{kc}")
                nc.scalar.activation(
                    e_t[:], st[:].rearrange("p (h n) -> p h n", h=2),
                    EXP, scale=float(SCALE))
                return e_t, mm0

            def emit_av(qb, p, kc, avs, e_t):
                for hh in range(2):
                    nc.tensor.matmul(
                        avs[hh][0:DIM_HEAD + 1, :],
                        v_sb[:, kc, 2 * p + hh, 0:DIM_HEAD + 1], e_t[:, hh, :],
                        start=(kc == 0), stop=(kc == N_KC - 1))

            def begin_qb(qb):
                den4 = stage.tile([128, 512], F32, tag="den4", name=f"den{qb}",
                                  bufs=1)
                nc.vector.memset(den4[:], 1.0)
                qb_state[qb] = dict(den4=den4)

            def evict_pair(qb, p, avs, tail=False):
                den4 = qb_state[qb]["den4"]
                avsb = []
                for hh in range(2):
                    a_sb = stage.tile([DIM_HEAD + 1, 512], F32, tag="avsb",
                                      name=f"avsb{qb}_{p}_{hh}", bufs=4)
                    k32 = 32 * (2 * p + hh)
                    if tail:
                        # ScalarE is exp-idle at the tail: evict O^T there
                        # while DVE pulls the denom rows straight from PSUM
                        nc.scalar.activation(
                            a_sb[:], avs[hh][0:DIM_HEAD + 1, :],
                            mybir.ActivationFunctionType.Copy)
                        nc.vector.tensor_copy(
                            den4[k32:k32 + 1, :],
                            avs[hh][DIM_HEAD:DIM_HEAD + 1, :])
                    else:
                        nc.vector.tensor_copy(a_sb[:],
                                              avs[hh][0:DIM_HEAD + 1, :])
                        nc.vector.tensor_copy(den4[k32:k32 + 1, :],
                                              a_sb[DIM_HEAD:DIM_HEAD + 1, :])
                    avsb.append(a_sb)
                return avsb

            def finalize_pair(qb, p, avsb, order_after=None, halves=1,
                              after_half=None):
                den4 = qb_state[qb]["den4"]
                qb0 = qb * QB
                rec = stage.tile([128, 512], F32, tag="rec",
                                 name=f"rec{qb}{p}", bufs=2)
                with nc.allow_low_precision(reason="softmax denom recip"):
                    nc.vector.reciprocal_approx_fast(rec[:], den4[:])
                recr = stage.tile([128, 512], F16, tag="recr",
                                  name=f"recr{qb}{p}", bufs=2)
                nc.vector.tensor_copy(recr[:], rec[:])
                bc = ps_misc.tile([128, 512], F32, tag="mp", name=f"bc{qb}{p}")
                hw = QB // halves
                for half in range(halves):
                    hs = slice(half * hw, (half + 1) * hw)
                    bcmm = nc.tensor.matmul(bc[:, hs], pat_sb[:, p, :],
                                            recr[:, hs], start=True, stop=True)
                    if order_after is not None:
                        add_dep_helper(order_after.ins, bcmm.ins, sync=False,
                                       reason="hold bc behind ST stream")
                for half in range(halves):
                    hs = slice(half * hw, (half + 1) * hw)
                    for hh in range(2):
                        nc.vector.tensor_mul(
                            ot_sb[hh * 64:(hh + 1) * 64, p,
                                  qb0 + half * hw:qb0 + (half + 1) * hw],
                            avsb[hh][0:DIM_HEAD, hs],
                            bc[hh * 64:(hh + 1) * 64, hs])
                    if after_half is not None:
                        after_half(half)

            def new_avs(qb, p):
                return [ps_av.tile([128, 512], F32, tag=f"av{hh}",
                                   name=f"av{hh}_{qb}_{p}")
                        for hh in range(2)]

            def emit_late_weights():
                nc.sync.dma_start(wo_sb[:],
                                  wo.rearrange("(c p) d -> p c d", p=128))
                nc.sync.dma_start(pat_sb[:], pat4[:])

            def phase_fillers(qb, p):
                f = []
                if qb == 0 and p == 0:
                    f.append((5, emit_late_weights))
                    for n in range(1, N_QB):
                        f.append((4 * n - 1, lambda n=n: (emit_kt(n),
                                                          emit_vblock(n))))
                elif qb == 0 and p == 1:
                    f.append((7, lambda: emit_qt(1)))
                else:
                    prev = qb - 1
                    if p == 0:
                        for g in range(4):
                            f.append(((9, 11, 13, 15)[g],
                                      lambda g=g: emit_outproj_chunk(prev, g)))
                    else:
                        if qb < N_QB - 1:
                            f.append((5, lambda: emit_qt(qb + 1)))
                        for g in range(4):
                            f.append(((3, 7, 10, 13)[g],
                                      lambda g=g: emit_outproj_chunk(prev, 4 + g)))
                return dict(f)

            # pre-issue every input-block DMA in consumption order through
            # a deep xin ring so transfers prefetch ahead of the chains
            # that consume them
            wv_sb = wpool.tile([128, 8, INNER], F16)
            xin_tiles = {}
            xin_order = [('k', 0), ('q', 0), ('v', 0), ('k', 1), ('v', 1),
                         ('k', 2), ('v', 2), ('k', 3), ('v', 3),
                         ('q', 1), ('q', 2), ('q', 3)]
            xin_src = {'k': xkt_r, 'q': xqt_r, 'v': xvt_r}
            for kind, n in xin_order:
                t = xin.tile([128, 8, QB], F16, tag="xin",
                             name=f"x{kind}_{n}", bufs=4)
                ns = slice(n * QB, (n + 1) * QB)
                nc.gpsimd.dma_start(t[:], xin_src[kind][:, :, ns])
                xin_tiles[(kind, n)] = t
                if (kind, n) == ('k', 0):
                    nc.scalar.dma_start(
                        wv_sb[:], wv.rearrange("(c p) m -> p c m", p=128))

            emit_kt(0)
            emit_qt(0)

            # ones column via memset — a DMA here degenerates to 64 tiny
            # 2-byte descriptors per partition and takes multiple us
            nc.vector.memset(v_sb[:, :, :, DIM_HEAD:DIM_HEAD + 1], 1.0)
            wo_sb = wpool.tile([128, 2, D_MODEL], F16)
            pat_sb = wpool.tile([128, 2, 128], F16)

            emit_vblock(0)

            AV_LAG = 4
            phases = [(qb, p) for qb in range(N_QB) for p in range(2)]
            pending = None      # (qb, p, avs, [(kc, e_t)...])
            pending_fin = None  # (qb, p, avsb)

            for qb, p in phases:
                if p == 0:
                    begin_qb(qb)
                avs = new_avs(qb, p)
                fillers = phase_fillers(qb, p)
                eq = []
                for kc in range(N_KC):
                    e_t, stmm = emit_st(qb, p, kc)
                    eq.append((kc, e_t))
                    if kc == AV_LAG - 1 and pending is not None:
                        pq, pp, pavs, peq = pending
                        for pkc, pe_t in peq:
                            emit_av(pq, pp, pkc, pavs, pe_t)
                        pending_fin = (pq, pp, evict_pair(pq, pp, pavs))
                        pending = None
                    if kc == 7 and pending_fin is not None:
                        fq, fp, favsb = pending_fin
                        finalize_pair(fq, fp, favsb, order_after=stmm)
                        pending_fin = None
                    if kc >= AV_LAG:
                        pkc, pe_t = eq[kc - AV_LAG]
                        emit_av(qb, p, pkc, avs, pe_t)
                    if kc in fillers:
                        fillers[kc]()
                pending = (qb, p, avs, eq[N_KC - AV_LAG:])

            pq, pp, pavs, peq = pending
            for pkc, pe_t in peq:
                emit_av(pq, pp, pkc, pavs, pe_t)
            # tail: normalize by query halves, interleaving the final
            # out-projection chunks so PE work overlaps the recip/mul chain
            finalize_pair(pq, pp, evict_pair(pq, pp, pavs, tail=True), halves=2,
                          after_half=lambda half: [
                              emit_outproj_chunk(N_QB - 1, 4 * half + g,
                                                 tail=True)
                              for g in range(4)])
    nc.compile()
    return nc


_NC_CACHE = None


def _get_nc():
    global _NC_CACHE
    if _NC_CACHE is None:
        _NC_CACHE = build_nc()
    return _NC_CACHE


def _make_pat4():
    pat = np.zeros((128, 2, 128), np.float16)
    for p in range(2):
        for hh in range(2):
            pat[32 * (2 * p + hh), p, hh * 64:(hh + 1) * 64] = 1.0
    return pat


def make_in_maps(query, key, value, Wq, Wk, Wv, Wo):
    query = np.asarray(query, np.float32)
    key = np.asarray(key, np.float32)
    value = np.asarray(value, np.float32)
    pat4 = _make_pat4()
    in_maps = []
    for c in range(N_CORES):
        b = c // 4
        hg = c % 4
        cols = slice(hg * INNER, (hg + 1) * INNER)
        in_maps.append({
            "xqt": np.ascontiguousarray(query[b].T).astype(np.float16),
            "xkt": np.ascontiguousarray(key[b].T).astype(np.float16),
            "xvt": np.ascontiguousarray(value[b].T).astype(np.float16),
            "wq": np.asarray(Wq[:, cols]).astype(np.float16),
            "wk": np.asarray(Wk[:, cols]).astype(np.float16),
            "wv": np.asarray(Wv[:, cols]).astype(np.float16),
            "wo": np.asarray(Wo[cols, :]).astype(np.float16),
            "pat4": pat4,
        })
    return in_maps


def kernel(query, key, value, Wq, Wk, Wv, Wo, bo, _trace=False, _trace_cores=None):
    nc = _get_nc()
    in_maps = make_in_maps(query, key, value, Wq, Wk, Wv, Wo)
    res = bass_utils.run_bass_kernel_spmd(
        nc, in_maps, core_ids=list(range(N_CORES)), trace=_trace,
        trace_cores=_trace_cores)
    out = np.zeros((B, N, D_MODEL), np.float32)
    for c in range(N_CORES):
        out[c // 4] += res.results[c]["out"].astype(np.float32)
    out += np.asarray(bo, np.float32)[None, None, :]
    if _trace:
        return out, res
    return out


# revision 59
# speedup vs baseline: 1.4629x; 1.1931x over previous
"""Multi-head attention (B=2, N=2048, d_model=1024, 16 heads x 64) on 8
Trainium2 NeuronCores.

Sharding: batch x head-group. Core c handles batch b = c//4 and heads
4*(c%4) .. 4*(c%4)+3. Projection weights are column-sliced (rows for Wo) so
each core computes q/k/v projections only for its 4 heads, full attention
for those heads, and a partial output projection. The host sums the four
partial outputs per batch (tensor-parallel reduce on to_out) and adds bo.

All matmul operands are fp16 (PSUM accumulate stays fp32): same PE
row-stream rate as fp32r but half the LDWEIGHTS install time, half the
DMA bytes, and 2x DVE rates on 16-bit evictions.

Device kernel (per core):
  warm  : dummy matmuls on a zero tile raise the PE out of its low
          p-state while the first input DMAs land (~12us first-DMA
          ring latency); all x-block DMAs are pre-issued on the gpsimd
          queue through a 3-deep ring so transfers prefetch in
          consumption order
  qT/kT : projections producing [head-dim, seq] (lhsT = W chunk)
  v     : natural [seq, head-dim] with a ones column folded in (M=65),
          K=128 contraction chunks; storage padded to 66 cols so every
          head slice starts 4-byte aligned (2-byte-aligned fp16
          LDWEIGHTS corrupts nondeterministically); ones written by
          memset (the strided DMA degenerates to 8K 2-byte descriptors)
  ST    : k^T q per head -> scores^T [keys, queries]; K=64 row-tile PAIRS
          (two heads on PE tiles T0/T8)
  E     : exp(ST * scale) via ScalarE eviction PSUM->SBUF -> fp16 (the
          wall: ~1 elem/lane/cycle regardless of dtype)
  AV    : [v|ones]^T @ E -> [65, 512]: rows 0-63 = O^T, row 64 = denom
  norm  : reciprocal_approx_fast (single custom-DVE op, ~5x faster than
          reciprocal) on den4, fp16 cast, K=4 pattern matmul broadcasts
          the per-query recip across the 64 output lanes, DVE multiply
          into O^T  (gpsimd partition_broadcast would be cheaper but
          silently produces garbage on HW in this compile path)
  out   : O^T-as-lhsT @ Wo slice -> partial [2048, 1024] written fp16;
          the last query block's finalize is split into query halves
          interleaved with its out-proj chunks, which run on the freed
          AV PSUM banks (4-slot ring) with ScalarE evictions and the
          final writes spread over all three DMA queues
"""

import numpy as np

import concourse.mybir as mybir
import concourse.tile as tile
from concourse import bacc
from concourse import bass_utils
from concourse.tile_rust import add_dep_helper

F32 = mybir.dt.float32
F16 = mybir.dt.float16
EXP = mybir.ActivationFunctionType.Exp

B = 2
N = 2048
D_MODEL = 1024
NHEAD = 16
DIM_HEAD = 64
SCALE = DIM_HEAD ** (-0.5)
N_CORES = 8
HEADS_PER_CORE = 4          # 2 pairs
INNER = HEADS_PER_CORE * DIM_HEAD  # 256

QB = 512                    # query block
N_QB = N // QB              # 4
N_KC = N // 128             # 16 key chunks


def build_nc():
    nc = bacc.Bacc("TRN2", target_bir_lowering=False, debug=False,
                   num_devices=N_CORES)
    xqt = nc.dram_tensor("xqt", [D_MODEL, N], F16, kind="ExternalInput").ap()
    xkt = nc.dram_tensor("xkt", [D_MODEL, N], F16, kind="ExternalInput").ap()
    xvt = nc.dram_tensor("xvt", [D_MODEL, N], F16, kind="ExternalInput").ap()
    wq = nc.dram_tensor("wq", [D_MODEL, INNER], F16, kind="ExternalInput").ap()
    wk = nc.dram_tensor("wk", [D_MODEL, INNER], F16, kind="ExternalInput").ap()
    wv = nc.dram_tensor("wv", [D_MODEL, INNER], F16, kind="ExternalInput").ap()
    wo = nc.dram_tensor("wo", [INNER, D_MODEL], F16, kind="ExternalInput").ap()
    # bc pattern: pat4[k, p, m] = 1 where head k owns output rows m in pair p
    pat4 = nc.dram_tensor("pat4", [128, 2, 128], F16, kind="ExternalInput").ap()
    out = nc.dram_tensor("out", [N, D_MODEL], F16, kind="ExternalOutput").ap()

    with tile.TileContext(nc) as tc:
        with (
            tc.tile_pool(name="wpool", bufs=1) as wpool,
            tc.tile_pool(name="persist", bufs=1) as persist,
            tc.tile_pool(name="xin", bufs=3) as xin,
            tc.tile_pool(name="ering", bufs=10) as ering,
            tc.tile_pool(name="stage", bufs=3) as stage,
            tc.tile_pool(name="ps_st", bufs=2, space="PSUM") as ps_st,
            tc.tile_pool(name="ps_av", bufs=1, space="PSUM") as ps_av,
            tc.tile_pool(name="ps_misc", bufs=2, space="PSUM") as ps_misc,
        ):
            # ---- weights on sync queue, ordered by first use ----
            wk_sb = wpool.tile([128, 8, INNER], F16)
            nc.sync.dma_start(wk_sb[:], wk.rearrange("(c p) m -> p c m", p=128))
            wq_sb = wpool.tile([128, 8, INNER], F16)
            nc.sync.dma_start(wq_sb[:], wq.rearrange("(c p) m -> p c m", p=128))

            # ---- PE p-state warmup on a zeroed tile while DMAs land ----
            warm = wpool.tile([128, 512], F16)
            nc.vector.memset(warm[:], 0.0)
            for i in range(12):
                wp = ps_misc.tile([128, 512], F32, tag="mp", name=f"warm{i}")
                nc.tensor.matmul(wp[:], warm[:, 0:128], warm[:],
                                 start=True, stop=True)
            for i in range(20):
                wp = ps_misc.tile([128, 256], F32, tag="mp",
                                  padded_shape=[128, 512], name=f"warms{i}")
                nc.tensor.matmul(wp[:], warm[:, 0:128], warm[:, 0:256],
                                 start=True, stop=True)

            qt_sb = persist.tile([128, 2, N], F16)
            kt_sb = persist.tile([128, 2, N], F16)
            # last dim padded to 66 so each head's slice starts 4-byte
            # aligned (66*2 = 132 bytes); the matmul AP still reads 65 cols
            v_sb = persist.tile([128, N_KC, HEADS_PER_CORE, DIM_HEAD + 2], F16)
            ot_sb = persist.tile([128, 2, N], F16)

            xqt_r = xqt.rearrange("(c p) n -> p c n", p=128)
            xkt_r = xkt.rearrange("(c p) n -> p c n", p=128)
            xvt_r = xvt.rearrange("(c p) n -> p c n", p=128)

            def emit_kt_m(xk_t, n, m):
                ns = slice(n * QB, (n + 1) * QB)
                pk = ps_misc.tile([128, QB], F32, tag="mp", name=f"pk{n}{m}")
                for c in range(8):
                    nc.tensor.matmul(
                        pk[:], wk_sb[:, c, m * 128:(m + 1) * 128],
                        xk_t[:, c, :], start=(c == 0), stop=(c == 7))
                nc.vector.tensor_copy(kt_sb[:, m, ns], pk[:])

            def emit_kt(n, ms=(0, 1)):
                xk_t = xin_tiles[('k', n)]
                for m in ms:
                    emit_kt_m(xk_t, n, m)
                return xk_t

            def emit_qt_m(xq_t, n, m):
                ns = slice(n * QB, (n + 1) * QB)
                pq = ps_misc.tile([128, QB], F32, tag="mp", name=f"pq{n}{m}")
                for c in range(8):
                    nc.tensor.matmul(
                        pq[:], wq_sb[:, c, m * 128:(m + 1) * 128],
                        xq_t[:, c, :], start=(c == 0), stop=(c == 7))
                nc.vector.tensor_copy(qt_sb[:, m, ns], pq[:])

            def emit_qt(n, ms=(0, 1)):
                xq_t = xin_tiles[('q', n)]
                for m in ms:
                    emit_qt_m(xq_t, n, m)
                return xq_t

            def emit_vblock(n):
                xv_t = xin_tiles[('v', n)]
                for kci in range(4):
                    kc = n * 4 + kci
                    kcs = slice(kci * 128, (kci + 1) * 128)
                    pv = ps_misc.tile([128, INNER], F32, tag="mp",
                                      padded_shape=[128, 512], name=f"pv{kc}")
                    for c in range(8):
                        nc.tensor.matmul(
                            pv[:], xv_t[:, c, kcs],
                            wv_sb[:, c, :], start=(c == 0), stop=(c == 7))
                    nc.vector.tensor_copy(
                        v_sb[:, kc, :, 0:DIM_HEAD],
                        pv[:].rearrange("p (h d) -> p h d", h=HEADS_PER_CORE))

            def emit_outproj_chunk(qb, idx, tail=False):
                qc = qb * 4 + idx // 2
                dc = idx % 2
                cs = slice(qc * 128, (qc + 1) * 128)
                if tail and idx % 4 >= 2:
                    # after the last evict_pair the AV PSUM banks are free:
                    # widen the tail ring to 4 slots so the out-projection
                    # matmuls don't serialize behind evictions
                    op = ps_av.tile([128, 512], F32, tag=f"av{idx % 2}",
                                    name=f"op{qc}{dc}")
                else:
                    op = ps_misc.tile([128, 512], F32, tag="mp",
                                      name=f"op{qc}{dc}")
                for ic in range(2):
                    nc.tensor.matmul(
                        op[:], ot_sb[:, ic, cs],
                        wo_sb[:, ic, dc * 512:(dc + 1) * 512],
                        start=(ic == 0), stop=(ic == 1))
                o_stage = stage.tile([128, 512], F16,
                                     tag="ostaget" if tail else "ostage",
                                     name=f"ost{qc}{dc}", bufs=4 if tail else 2)
                if tail:
                    nc.scalar.activation(o_stage[:], op[:],
                                         mybir.ActivationFunctionType.Copy)
                    # spread the final writes over all three DMA queues so
                    # the drain isn't serialized on one ring
                    q = (nc.sync, nc.scalar, nc.gpsimd)[idx % 3]
                    q.dma_start(out[cs, dc * 512:(dc + 1) * 512], o_stage[:])
                else:
                    nc.vector.tensor_copy(o_stage[:], op[:])
                    nc.sync.dma_start(out[cs, dc * 512:(dc + 1) * 512],
                                      o_stage[:])

            qb_state = {}

            def emit_st(qb, p, kc):
                qs = slice(qb * QB, (qb + 1) * QB)
                ks = slice(kc * 128, (kc + 1) * 128)
                st = ps_st.tile([128, 1024], F32, tag="st", name=f"st{qb}{p}{kc}")
                mm0 = nc.tensor.matmul(st[:, 0:512], kt_sb[0:64, p, ks],
                                       qt_sb[0:64, p, qs], start=True, stop=True)
                nc.tensor.matmul(st[:, 512:1024], kt_sb[64:128, p, ks],
                                 qt_sb[64:128, p, qs], start=True, stop=True)
                e_t = ering.tile([128, 2, 512], F16, tag="e",
                                 name=f"e{qb}{p}{kc}")
                nc.scalar.activation(
                    e_t[:], st[:].rearrange("p (h n) -> p h n", h=2),
                    EXP, scale=float(SCALE))
                return e_t, mm0

            def emit_av(qb, p, kc, avs, e_t):
                for hh in range(2):
                    nc.tensor.matmul(
                        avs[hh][0:DIM_HEAD + 1, :],
                        v_sb[:, kc, 2 * p + hh, 0:DIM_HEAD + 1], e_t[:, hh, :],
                        start=(kc == 0), stop=(kc == N_KC - 1))

            def begin_qb(qb):
                den4 = stage.tile([128, 512], F32, tag="den4", name=f"den{qb}",
                                  bufs=1)
                nc.vector.memset(den4[:], 1.0)
                qb_state[qb] = dict(den4=den4)

            def evict_pair(qb, p, avs, tail=False):
                den4 = qb_state[qb]["den4"]
                avsb = []
                for hh in range(2):
                    a_sb = stage.tile([DIM_HEAD + 1, 512], F32, tag="avsb",
                                      name=f"avsb{qb}_{p}_{hh}", bufs=4)
                    k32 = 32 * (2 * p + hh)
                    if tail:
                        # ScalarE is exp-idle at the tail: evict O^T there
                        # while DVE pulls the denom rows straight from PSUM
                        nc.scalar.activation(
                            a_sb[:], avs[hh][0:DIM_HEAD + 1, :],
                            mybir.ActivationFunctionType.Copy)
                        nc.vector.tensor_copy(
                            den4[k32:k32 + 1, :],
                            avs[hh][DIM_HEAD:DIM_HEAD + 1, :])
                    else:
                        nc.vector.tensor_copy(a_sb[:],
                                              avs[hh][0:DIM_HEAD + 1, :])
                        nc.vector.tensor_copy(den4[k32:k32 + 1, :],
                                              a_sb[DIM_HEAD:DIM_HEAD + 1, :])
                    avsb.append(a_sb)
                return avsb

            def finalize_pair(qb, p, avsb, order_after=None, halves=1,
                              after_half=None):
                den4 = qb_state[qb]["den4"]
                qb0 = qb * QB
                rec = stage.tile([128, 512], F32, tag="rec",
                                 name=f"rec{qb}{p}", bufs=2)
                with nc.allow_low_precision(reason="softmax denom recip"):
                    nc.vector.reciprocal_approx_fast(rec[:], den4[:])
                recr = stage.tile([128, 512], F16, tag="recr",
                                  name=f"recr{qb}{p}", bufs=2)
                nc.vector.tensor_copy(recr[:], rec[:])
                bc = ps_misc.tile([128, 512], F32, tag="mp", name=f"bc{qb}{p}")
                hw = QB // halves
                for half in range(halves):
                    hs = slice(half * hw, (half + 1) * hw)
                    bcmm = nc.tensor.matmul(bc[:, hs], pat_sb[:, p, :],
                                            recr[:, hs], start=True, stop=True)
                    if order_after is not None:
                        add_dep_helper(order_after.ins, bcmm.ins, sync=False,
                                       reason="hold bc behind ST stream")
                for half in range(halves):
                    hs = slice(half * hw, (half + 1) * hw)
                    for hh in range(2):
                        nc.vector.tensor_mul(
                            ot_sb[hh * 64:(hh + 1) * 64, p,
                                  qb0 + half * hw:qb0 + (half + 1) * hw],
                            avsb[hh][0:DIM_HEAD, hs],
                            bc[hh * 64:(hh + 1) * 64, hs])
                    if after_half is not None:
                        after_half(half)

            def new_avs(qb, p):
                return [ps_av.tile([128, 512], F32, tag=f"av{hh}",
                                   name=f"av{hh}_{qb}_{p}")
                        for hh in range(2)]

            def emit_late_weights():
                nc.sync.dma_start(wo_sb[:],
                                  wo.rearrange("(c p) d -> p c d", p=128))
                nc.sync.dma_start(pat_sb[:], pat4[:])

            def phase_fillers(qb, p):
                f = []
                if qb == 0 and p == 0:
                    f.append((5, emit_late_weights))
                    for n in range(1, N_QB):
                        f.append((4 * n - 1, lambda n=n: (emit_kt(n),
                                                          emit_vblock(n))))
                elif qb == 0 and p == 1:
                    f.append((7, lambda: emit_qt(1)))
                else:
                    prev = qb - 1
                    if p == 0:
                        for g in range(4):
                            f.append(((9, 11, 13, 15)[g],
                                      lambda g=g: emit_outproj_chunk(prev, g)))
                    else:
                        if qb < N_QB - 1:
                            f.append((5, lambda: emit_qt(qb + 1)))
                        for g in range(4):
                            f.append(((3, 7, 10, 13)[g],
                                      lambda g=g: emit_outproj_chunk(prev, 4 + g)))
                return dict(f)

            # pre-issue every input-block DMA in consumption order through
            # a deep xin ring so transfers prefetch ahead of the chains
            # that consume them
            wv_sb = wpool.tile([128, 8, INNER], F16)
            xin_tiles = {}
            xin_order = [('k', 0), ('q', 0), ('v', 0), ('k', 1), ('v', 1),
                         ('k', 2), ('v', 2), ('k', 3), ('v', 3),
                         ('q', 1), ('q', 2), ('q', 3)]
            xin_src = {'k': xkt_r, 'q': xqt_r, 'v': xvt_r}
            for kind, n in xin_order:
                t = xin.tile([128, 8, QB], F16, tag="xin",
                             name=f"x{kind}_{n}", bufs=4)
                ns = slice(n * QB, (n + 1) * QB)
                nc.gpsimd.dma_start(t[:], xin_src[kind][:, :, ns])
                xin_tiles[(kind, n)] = t
                if (kind, n) == ('k', 0):
                    nc.scalar.dma_start(
                        wv_sb[:], wv.rearrange("(c p) m -> p c m", p=128))

            emit_kt(0)
            emit_qt(0)

            # ones column via memset — a DMA here degenerates to 64 tiny
            # 2-byte descriptors per partition and takes multiple us
            nc.vector.memset(v_sb[:, :, :, DIM_HEAD:DIM_HEAD + 1], 1.0)
            wo_sb = wpool.tile([128, 2, D_MODEL], F16)
            pat_sb = wpool.tile([128, 2, 128], F16)

            emit_vblock(0)

            AV_LAG = 5
            phases = [(qb, p) for qb in range(N_QB) for p in range(2)]
            pending = None      # (qb, p, avs, [(kc, e_t)...])
            pending_fin = None  # (qb, p, avsb)

            for qb, p in phases:
                if p == 0:
                    begin_qb(qb)
                avs = new_avs(qb, p)
                fillers = phase_fillers(qb, p)
                eq = []
                for kc in range(N_KC):
                    e_t, stmm = emit_st(qb, p, kc)
                    eq.append((kc, e_t))
                    if kc == AV_LAG - 1 and pending is not None:
                        pq, pp, pavs, peq = pending
                        for pkc, pe_t in peq:
                            emit_av(pq, pp, pkc, pavs, pe_t)
                        pending_fin = (pq, pp, evict_pair(pq, pp, pavs))
                        pending = None
                    if kc == 7 and pending_fin is not None:
                        fq, fp, favsb = pending_fin
                        finalize_pair(fq, fp, favsb, order_after=stmm)
                        pending_fin = None
                    if kc >= AV_LAG:
                        pkc, pe_t = eq[kc - AV_LAG]
                        emit_av(qb, p, pkc, avs, pe_t)
                    if kc in fillers:
                        fillers[kc]()
                pending = (qb, p, avs, eq[N_KC - AV_LAG:])

            pq, pp, pavs, peq = pending
            for pkc, pe_t in peq:
                emit_av(pq, pp, pkc, pavs, pe_t)
            # tail: normalize by query halves, interleaving the final
            # out-projection chunks so PE work overlaps the recip/mul chain
            finalize_pair(pq, pp, evict_pair(pq, pp, pavs, tail=True), halves=2,
                          after_half=lambda half: [
                              emit_outproj_chunk(N_QB - 1, 4 * half + g,
                                                 tail=True)
                              for g in range(4)])
    nc.compile()
    return nc


_NC_CACHE = None


def _get_nc():
    global _NC_CACHE
    if _NC_CACHE is None:
        _NC_CACHE = build_nc()
    return _NC_CACHE


def _make_pat4():
    pat = np.zeros((128, 2, 128), np.float16)
    for p in range(2):
        for hh in range(2):
            pat[32 * (2 * p + hh), p, hh * 64:(hh + 1) * 64] = 1.0
    return pat


def make_in_maps(query, key, value, Wq, Wk, Wv, Wo):
    query = np.asarray(query, np.float32)
    key = np.asarray(key, np.float32)
    value = np.asarray(value, np.float32)
    pat4 = _make_pat4()
    in_maps = []
    for c in range(N_CORES):
        b = c // 4
        hg = c % 4
        cols = slice(hg * INNER, (hg + 1) * INNER)
        in_maps.append({
            "xqt": np.ascontiguousarray(query[b].T).astype(np.float16),
            "xkt": np.ascontiguousarray(key[b].T).astype(np.float16),
            "xvt": np.ascontiguousarray(value[b].T).astype(np.float16),
            "wq": np.asarray(Wq[:, cols]).astype(np.float16),
            "wk": np.asarray(Wk[:, cols]).astype(np.float16),
            "wv": np.asarray(Wv[:, cols]).astype(np.float16),
            "wo": np.asarray(Wo[cols, :]).astype(np.float16),
            "pat4": pat4,
        })
    return in_maps


def kernel(query, key, value, Wq, Wk, Wv, Wo, bo, _trace=False, _trace_cores=None):
    nc = _get_nc()
    in_maps = make_in_maps(query, key, value, Wq, Wk, Wv, Wo)
    res = bass_utils.run_bass_kernel_spmd(
        nc, in_maps, core_ids=list(range(N_CORES)), trace=_trace,
        trace_cores=_trace_cores)
    out = np.zeros((B, N, D_MODEL), np.float32)
    for c in range(N_CORES):
        out[c // 4] += res.results[c]["out"].astype(np.float32)
    out += np.asarray(bo, np.float32)[None, None, :]
    if _trace:
        return out, res
    return out


# revision 60
# speedup vs baseline: 1.4689x; 1.0041x over previous
"""Multi-head attention (B=2, N=2048, d_model=1024, 16 heads x 64) on 8
Trainium2 NeuronCores.

Sharding: batch x head-group. Core c handles batch b = c//4 and heads
4*(c%4) .. 4*(c%4)+3. Projection weights are column-sliced (rows for Wo) so
each core computes q/k/v projections only for its 4 heads, full attention
for those heads, and a partial output projection. The host sums the four
partial outputs per batch (tensor-parallel reduce on to_out) and adds bo.

All matmul operands are fp16 (PSUM accumulate stays fp32): same PE
row-stream rate as fp32r but half the LDWEIGHTS install time, half the
DMA bytes, and 2x DVE rates on 16-bit evictions.

Device kernel (per core):
  warm  : dummy matmuls on a zero tile raise the PE out of its low
          p-state while the first input DMAs land (~12us first-DMA
          ring latency); all x-block DMAs are pre-issued on the gpsimd
          queue through a 3-deep ring so transfers prefetch in
          consumption order
  qT/kT : projections producing [head-dim, seq] (lhsT = W chunk)
  v     : natural [seq, head-dim] with a ones column folded in (M=65),
          K=128 contraction chunks; storage padded to 66 cols so every
          head slice starts 4-byte aligned (2-byte-aligned fp16
          LDWEIGHTS corrupts nondeterministically); ones written by
          memset (the strided DMA degenerates to 8K 2-byte descriptors)
  ST    : k^T q per head -> scores^T [keys, queries]; K=64 row-tile PAIRS
          (two heads on PE tiles T0/T8)
  E     : exp(ST * scale) via ScalarE eviction PSUM->SBUF -> fp16 (the
          wall: ~1 elem/lane/cycle regardless of dtype)
  AV    : [v|ones]^T @ E -> [65, 512]: rows 0-63 = O^T, row 64 = denom
  norm  : reciprocal_approx_fast (single custom-DVE op, ~5x faster than
          reciprocal) on den4, fp16 cast, K=4 pattern matmul broadcasts
          the per-query recip across the 64 output lanes, DVE multiply
          into O^T  (gpsimd partition_broadcast would be cheaper but
          silently produces garbage on HW in this compile path)
  out   : O^T-as-lhsT @ Wo slice -> partial [2048, 1024] written fp16;
          the last query block's finalize is split into query halves
          interleaved with its out-proj chunks, which run on the freed
          AV PSUM banks (4-slot ring) with ScalarE evictions and the
          final writes spread over all three DMA queues
"""

import numpy as np

import concourse.mybir as mybir
import concourse.tile as tile
from concourse import bacc
from concourse import bass_utils
from concourse.tile_rust import add_dep_helper

F32 = mybir.dt.float32
F16 = mybir.dt.float16
EXP = mybir.ActivationFunctionType.Exp

B = 2
N = 2048
D_MODEL = 1024
NHEAD = 16
DIM_HEAD = 64
SCALE = DIM_HEAD ** (-0.5)
N_CORES = 8
HEADS_PER_CORE = 4          # 2 pairs
INNER = HEADS_PER_CORE * DIM_HEAD  # 256

QB = 512                    # query block
N_QB = N // QB              # 4
N_KC = N // 128             # 16 key chunks


def build_nc():
    nc = bacc.Bacc("TRN2", target_bir_lowering=False, debug=False,
                   num_devices=N_CORES)
    xqt = nc.dram_tensor("xqt", [D_MODEL, N], F16, kind="ExternalInput").ap()
    xkt = nc.dram_tensor("xkt", [D_MODEL, N], F16, kind="ExternalInput").ap()
    xvt = nc.dram_tensor("xvt", [D_MODEL, N], F16, kind="ExternalInput").ap()
    wq = nc.dram_tensor("wq", [D_MODEL, INNER], F16, kind="ExternalInput").ap()
    wk = nc.dram_tensor("wk", [D_MODEL, INNER], F16, kind="ExternalInput").ap()
    wv = nc.dram_tensor("wv", [D_MODEL, INNER], F16, kind="ExternalInput").ap()
    wo = nc.dram_tensor("wo", [INNER, D_MODEL], F16, kind="ExternalInput").ap()
    # bc pattern: pat4[k, p, m] = 1 where head k owns output rows m in pair p
    pat4 = nc.dram_tensor("pat4", [128, 2, 128], F16, kind="ExternalInput").ap()
    out = nc.dram_tensor("out", [N, D_MODEL], F16, kind="ExternalOutput").ap()

    with tile.TileContext(nc) as tc:
        with (
            tc.tile_pool(name="wpool", bufs=1) as wpool,
            tc.tile_pool(name="persist", bufs=1) as persist,
            tc.tile_pool(name="xin", bufs=3) as xin,
            tc.tile_pool(name="ering", bufs=9) as ering,
            tc.tile_pool(name="stage", bufs=3) as stage,
            tc.tile_pool(name="ps_st", bufs=2, space="PSUM") as ps_st,
            tc.tile_pool(name="ps_av", bufs=1, space="PSUM") as ps_av,
            tc.tile_pool(name="ps_misc", bufs=2, space="PSUM") as ps_misc,
        ):
            # ---- weights on sync queue, ordered by first use ----
            wk_sb = wpool.tile([128, 8, INNER], F16)
            nc.sync.dma_start(wk_sb[:], wk.rearrange("(c p) m -> p c m", p=128))
            wq_sb = wpool.tile([128, 8, INNER], F16)
            nc.sync.dma_start(wq_sb[:], wq.rearrange("(c p) m -> p c m", p=128))

            # ---- PE p-state warmup on a zeroed tile while DMAs land ----
            warm = wpool.tile([128, 512], F16)
            nc.vector.memset(warm[:], 0.0)
            for i in range(12):
                wp = ps_misc.tile([128, 512], F32, tag="mp", name=f"warm{i}")
                nc.tensor.matmul(wp[:], warm[:, 0:128], warm[:],
                                 start=True, stop=True)
            for i in range(20):
                wp = ps_misc.tile([128, 256], F32, tag="mp",
                                  padded_shape=[128, 512], name=f"warms{i}")
                nc.tensor.matmul(wp[:], warm[:, 0:128], warm[:, 0:256],
                                 start=True, stop=True)

            qt_sb = persist.tile([128, 2, N], F16)
            kt_sb = persist.tile([128, 2, N], F16)
            # last dim padded to 66 so each head's slice starts 4-byte
            # aligned (66*2 = 132 bytes); the matmul AP still reads 65 cols
            v_sb = persist.tile([128, N_KC, HEADS_PER_CORE, DIM_HEAD + 2], F16)
            ot_sb = persist.tile([128, 2, N], F16)

            xqt_r = xqt.rearrange("(c p) n -> p c n", p=128)
            xkt_r = xkt.rearrange("(c p) n -> p c n", p=128)
            xvt_r = xvt.rearrange("(c p) n -> p c n", p=128)

            def emit_kt_m(xk_t, n, m):
                ns = slice(n * QB, (n + 1) * QB)
                pk = ps_misc.tile([128, QB], F32, tag="mp", name=f"pk{n}{m}")
                for c in range(8):
                    nc.tensor.matmul(
                        pk[:], wk_sb[:, c, m * 128:(m + 1) * 128],
                        xk_t[:, c, :], start=(c == 0), stop=(c == 7))
                nc.vector.tensor_copy(kt_sb[:, m, ns], pk[:])

            def emit_kt(n, ms=(0, 1)):
                xk_t = xin_tiles[('k', n)]
                for m in ms:
                    emit_kt_m(xk_t, n, m)
                return xk_t

            def emit_qt_m(xq_t, n, m):
                ns = slice(n * QB, (n + 1) * QB)
                pq = ps_misc.tile([128, QB], F32, tag="mp", name=f"pq{n}{m}")
                for c in range(8):
                    nc.tensor.matmul(
                        pq[:], wq_sb[:, c, m * 128:(m + 1) * 128],
                        xq_t[:, c, :], start=(c == 0), stop=(c == 7))
                nc.vector.tensor_copy(qt_sb[:, m, ns], pq[:])

            def emit_qt(n, ms=(0, 1)):
                xq_t = xin_tiles[('q', n)]
                for m in ms:
                    emit_qt_m(xq_t, n, m)
                return xq_t

            def emit_vblock(n):
                xv_t = xin_tiles[('v', n)]
                for kci in range(4):
                    kc = n * 4 + kci
                    kcs = slice(kci * 128, (kci + 1) * 128)
                    pv = ps_misc.tile([128, INNER], F32, tag="mp",
                                      padded_shape=[128, 512], name=f"pv{kc}")
                    for c in range(8):
                        nc.tensor.matmul(
                            pv[:], xv_t[:, c, kcs],
                            wv_sb[:, c, :], start=(c == 0), stop=(c == 7))
                    nc.vector.tensor_copy(
                        v_sb[:, kc, :, 0:DIM_HEAD],
                        pv[:].rearrange("p (h d) -> p h d", h=HEADS_PER_CORE))

            def emit_outproj_chunk(qb, idx, tail=False):
                qc = qb * 4 + idx // 2
                dc = idx % 2
                cs = slice(qc * 128, (qc + 1) * 128)
                if tail and idx % 4 >= 2:
                    # after the last evict_pair the AV PSUM banks are free:
                    # widen the tail ring to 4 slots so the out-projection
                    # matmuls don't serialize behind evictions
                    op = ps_av.tile([128, 512], F32, tag=f"av{idx % 2}",
                                    name=f"op{qc}{dc}")
                else:
                    op = ps_misc.tile([128, 512], F32, tag="mp",
                                      name=f"op{qc}{dc}")
                for ic in range(2):
                    nc.tensor.matmul(
                        op[:], ot_sb[:, ic, cs],
                        wo_sb[:, ic, dc * 512:(dc + 1) * 512],
                        start=(ic == 0), stop=(ic == 1))
                o_stage = stage.tile([128, 512], F16,
                                     tag="ostaget" if tail else "ostage",
                                     name=f"ost{qc}{dc}", bufs=4 if tail else 2)
                if tail:
                    nc.scalar.activation(o_stage[:], op[:],
                                         mybir.ActivationFunctionType.Copy)
                    # spread the final writes over all three DMA queues so
                    # the drain isn't serialized on one ring
                    q = (nc.sync, nc.scalar, nc.gpsimd)[idx % 3]
                    q.dma_start(out[cs, dc * 512:(dc + 1) * 512], o_stage[:])
                else:
                    nc.vector.tensor_copy(o_stage[:], op[:])
                    nc.sync.dma_start(out[cs, dc * 512:(dc + 1) * 512],
                                      o_stage[:])

            qb_state = {}

            def emit_st(qb, p, kc):
                qs = slice(qb * QB, (qb + 1) * QB)
                ks = slice(kc * 128, (kc + 1) * 128)
                st = ps_st.tile([128, 1024], F32, tag="st", name=f"st{qb}{p}{kc}")
                mm0 = nc.tensor.matmul(st[:, 0:512], kt_sb[0:64, p, ks],
                                       qt_sb[0:64, p, qs], start=True, stop=True)
                nc.tensor.matmul(st[:, 512:1024], kt_sb[64:128, p, ks],
                                 qt_sb[64:128, p, qs], start=True, stop=True)
                e_t = ering.tile([128, 2, 512], F16, tag="e",
                                 name=f"e{qb}{p}{kc}")
                nc.scalar.activation(
                    e_t[:], st[:].rearrange("p (h n) -> p h n", h=2),
                    EXP, scale=float(SCALE))
                return e_t, mm0

            def emit_av(qb, p, kc, avs, e_t):
                for hh in range(2):
                    nc.tensor.matmul(
                        avs[hh][0:DIM_HEAD + 1, :],
                        v_sb[:, kc, 2 * p + hh, 0:DIM_HEAD + 1], e_t[:, hh, :],
                        start=(kc == 0), stop=(kc == N_KC - 1))

            def begin_qb(qb):
                den4 = stage.tile([128, 512], F32, tag="den4", name=f"den{qb}",
                                  bufs=1)
                nc.vector.memset(den4[:], 1.0)
                qb_state[qb] = dict(den4=den4)

            def evict_pair(qb, p, avs, tail=False):
                den4 = qb_state[qb]["den4"]
                avsb = []
                for hh in range(2):
                    a_sb = stage.tile([DIM_HEAD + 1, 512], F32, tag="avsb",
                                      name=f"avsb{qb}_{p}_{hh}", bufs=4)
                    k32 = 32 * (2 * p + hh)
                    if tail:
                        # ScalarE is exp-idle at the tail: evict O^T there
                        # while DVE pulls the denom rows straight from PSUM
                        nc.scalar.activation(
                            a_sb[:], avs[hh][0:DIM_HEAD + 1, :],
                            mybir.ActivationFunctionType.Copy)
                        nc.vector.tensor_copy(
                            den4[k32:k32 + 1, :],
                            avs[hh][DIM_HEAD:DIM_HEAD + 1, :])
                    else:
                        nc.vector.tensor_copy(a_sb[:],
                                              avs[hh][0:DIM_HEAD + 1, :])
                        nc.vector.tensor_copy(den4[k32:k32 + 1, :],
                                              a_sb[DIM_HEAD:DIM_HEAD + 1, :])
                    avsb.append(a_sb)
                return avsb

            def finalize_pair(qb, p, avsb, order_after=None, halves=1,
                              after_half=None):
                den4 = qb_state[qb]["den4"]
                qb0 = qb * QB
                rec = stage.tile([128, 512], F32, tag="rec",
                                 name=f"rec{qb}{p}", bufs=2)
                with nc.allow_low_precision(reason="softmax denom recip"):
                    nc.vector.reciprocal_approx_fast(rec[:], den4[:])
                recr = stage.tile([128, 512], F16, tag="recr",
                                  name=f"recr{qb}{p}", bufs=2)
                nc.vector.tensor_copy(recr[:], rec[:])
                bc = ps_misc.tile([128, 512], F32, tag="mp", name=f"bc{qb}{p}")
                hw = QB // halves
                for half in range(halves):
                    hs = slice(half * hw, (half + 1) * hw)
                    bcmm = nc.tensor.matmul(bc[:, hs], pat_sb[:, p, :],
                                            recr[:, hs], start=True, stop=True)
                    if order_after is not None:
                        add_dep_helper(order_after.ins, bcmm.ins, sync=False,
                                       reason="hold bc behind ST stream")
                for half in range(halves):
                    hs = slice(half * hw, (half + 1) * hw)
                    for hh in range(2):
                        nc.vector.tensor_mul(
                            ot_sb[hh * 64:(hh + 1) * 64, p,
                                  qb0 + half * hw:qb0 + (half + 1) * hw],
                            avsb[hh][0:DIM_HEAD, hs],
                            bc[hh * 64:(hh + 1) * 64, hs])
                    if after_half is not None:
                        after_half(half)

            def new_avs(qb, p):
                return [ps_av.tile([128, 512], F32, tag=f"av{hh}",
                                   name=f"av{hh}_{qb}_{p}")
                        for hh in range(2)]

            def emit_late_weights():
                nc.sync.dma_start(wo_sb[:],
                                  wo.rearrange("(c p) d -> p c d", p=128))
                nc.sync.dma_start(pat_sb[:], pat4[:])

            def phase_fillers(qb, p):
                f = []
                if qb == 0 and p == 0:
                    f.append((5, emit_late_weights))
                    for n in range(1, N_QB):
                        f.append((4 * n - 1, lambda n=n: (emit_kt(n),
                                                          emit_vblock(n))))
                elif qb == 0 and p == 1:
                    f.append((7, lambda: emit_qt(1)))
                else:
                    prev = qb - 1
                    if p == 0:
                        for g in range(4):
                            f.append(((9, 11, 13, 15)[g],
                                      lambda g=g: emit_outproj_chunk(prev, g)))
                    else:
                        if qb < N_QB - 1:
                            f.append((5, lambda: emit_qt(qb + 1)))
                        for g in range(4):
                            f.append(((3, 7, 10, 13)[g],
                                      lambda g=g: emit_outproj_chunk(prev, 4 + g)))
                return dict(f)

            # pre-issue every input-block DMA in consumption order through
            # a deep xin ring so transfers prefetch ahead of the chains
            # that consume them
            wv_sb = wpool.tile([128, 8, INNER], F16)
            xin_tiles = {}
            xin_order = [('k', 0), ('q', 0), ('v', 0), ('k', 1), ('v', 1),
                         ('k', 2), ('v', 2), ('k', 3), ('v', 3),
                         ('q', 1), ('q', 2), ('q', 3)]
            xin_src = {'k': xkt_r, 'q': xqt_r, 'v': xvt_r}
            for kind, n in xin_order:
                t = xin.tile([128, 8, QB], F16, tag="xin",
                             name=f"x{kind}_{n}", bufs=4)
                ns = slice(n * QB, (n + 1) * QB)
                nc.gpsimd.dma_start(t[:], xin_src[kind][:, :, ns])
                xin_tiles[(kind, n)] = t
                if (kind, n) == ('k', 0):
                    nc.scalar.dma_start(
                        wv_sb[:], wv.rearrange("(c p) m -> p c m", p=128))

            emit_kt(0)
            emit_qt(0)

            # ones column via memset — a DMA here degenerates to 64 tiny
            # 2-byte descriptors per partition and takes multiple us
            nc.vector.memset(v_sb[:, :, :, DIM_HEAD:DIM_HEAD + 1], 1.0)
            wo_sb = wpool.tile([128, 2, D_MODEL], F16)
            pat_sb = wpool.tile([128, 2, 128], F16)

            emit_vblock(0)

            AV_LAG = 4
            phases = [(qb, p) for qb in range(N_QB) for p in range(2)]
            pending = None      # (qb, p, avs, [(kc, e_t)...])
            pending_fin = None  # (qb, p, avsb)

            for qb, p in phases:
                if p == 0:
                    begin_qb(qb)
                avs = new_avs(qb, p)
                fillers = phase_fillers(qb, p)
                eq = []
                for kc in range(N_KC):
                    e_t, stmm = emit_st(qb, p, kc)
                    eq.append((kc, e_t))
                    if kc == AV_LAG - 1 and pending is not None:
                        pq, pp, pavs, peq = pending
                        for pkc, pe_t in peq:
                            emit_av(pq, pp, pkc, pavs, pe_t)
                        pending_fin = (pq, pp, evict_pair(pq, pp, pavs))
                        pending = None
                    if kc == 7 and pending_fin is not None:
                        fq, fp, favsb = pending_fin
                        finalize_pair(fq, fp, favsb, order_after=stmm)
                        pending_fin = None
                    if kc >= AV_LAG:
                        pkc, pe_t = eq[kc - AV_LAG]
                        emit_av(qb, p, pkc, avs, pe_t)
                    if kc in fillers:
                        fillers[kc]()
                pending = (qb, p, avs, eq[N_KC - AV_LAG:])

            pq, pp, pavs, peq = pending
            for pkc, pe_t in peq:
                emit_av(pq, pp, pkc, pavs, pe_t)
            # tail: normalize by query halves, interleaving the final
            # out-projection chunks so PE work overlaps the recip/mul chain
            finalize_pair(pq, pp, evict_pair(pq, pp, pavs, tail=True), halves=2,
                          after_half=lambda half: [
                              emit_outproj_chunk(N_QB - 1, 4 * half + g,
                                                 tail=True)
                              for g in range(4)])
    nc.compile()
    return nc


_NC_CACHE = None


def _get_nc():
    global _NC_CACHE
    if _NC_CACHE is None:
        _NC_CACHE = build_nc()
    return _NC_CACHE


def _make_pat4():
    pat = np.zeros((128, 2, 128), np.float16)
    for p in range(2):
        for hh in range(2):
            pat[32 * (2 * p + hh), p, hh * 64:(hh + 1) * 64] = 1.0
    return pat


def make_in_maps(query, key, value, Wq, Wk, Wv, Wo):
    query = np.asarray(query, np.float32)
    key = np.asarray(key, np.float32)
    value = np.asarray(value, np.float32)
    pat4 = _make_pat4()
    in_maps = []
    for c in range(N_CORES):
        b = c // 4
        hg = c % 4
        cols = slice(hg * INNER, (hg + 1) * INNER)
        in_maps.append({
            "xqt": np.ascontiguousarray(query[b].T).astype(np.float16),
            "xkt": np.ascontiguousarray(key[b].T).astype(np.float16),
            "xvt": np.ascontiguousarray(value[b].T).astype(np.float16),
            "wq": np.asarray(Wq[:, cols]).astype(np.float16),
            "wk": np.asarray(Wk[:, cols]).astype(np.float16),
            "wv": np.asarray(Wv[:, cols]).astype(np.float16),
            "wo": np.asarray(Wo[cols, :]).astype(np.float16),
            "pat4": pat4,
        })
    return in_maps


def kernel(query, key, value, Wq, Wk, Wv, Wo, bo, _trace=False, _trace_cores=None):
    nc = _get_nc()
    in_maps = make_in_maps(query, key, value, Wq, Wk, Wv, Wo)
    res = bass_utils.run_bass_kernel_spmd(
        nc, in_maps, core_ids=list(range(N_CORES)), trace=_trace,
        trace_cores=_trace_cores)
    out = np.zeros((B, N, D_MODEL), np.float32)
    for c in range(N_CORES):
        out[c // 4] += res.results[c]["out"].astype(np.float32)
    out += np.asarray(bo, np.float32)[None, None, :]
    if _trace:
        return out, res
    return out
